# revision 49
# baseline (speedup 1.0000x reference)
"""Trainium2 Bass kernel for nn_MCA_12214886990440 (strip-conv dual-axis attention).

Sharding: data-parallel over batch B=8 across 8 NeuronCores (params replicated).

Per-core math (one batch element, C=64, H=W=128, NH=8, D=8):
  xh = bn1(x); xw = bn2(x)
  sc_h = 21-tap strip conv along H (3 kernel sizes presummed)
  sc_w = 21-tap strip conv along W
  qkv (conv bias folded into qkv bias); attention per head reassociated:
      w_o^T = wk_t @ G_w,  G_w = sum_d hq(d)^T wv(d)   [128x128 Gram]
      h_o^T = hk_t @ G_h,  G_h = sum_d wq(d)^T hv(d)
  y = x * sigmoid(wout@w_o + hout@h_o + b)

All matmuls run in fp16 (fp32 PSUM accumulate). Conv uses dual pairing:
K=128 packs taps (t, t+1) via a row/col-shifted image copy on partitions
64:128; M=128 packs two tap-pairs, either across adjacent 4-row output
chunks (h-branch) or within a widened 132-column window (w-branch). The
resulting partition-split partial sums are combined for free inside the
qkv matmuls by duplicating the qkv weights across K=128.

Layout pivots go through small DRAM tensors with DMA-friendly layouts:
q/v/k/s are parked as [pixel-row, (channel, pixel-col)] so every
attention-side load is a contiguous [128 x 2KB] transfer.
"""
import sys
sys.path.insert(0, "/opt/trn_rl_repo")

import numpy as np

import concourse.bass as bass
import concourse.tile as tile
from concourse import bacc
from concourse import mybir

B, C, H, W, NH, D = 8, 64, 128, 128, 8, 8
KS = [7, 11, 21]
EPS = 1e-5
PAD = 10          # max k//2
HW = H * W        # 16384
PROWS = H + 2 * PAD            # 148 padded rows (h-branch)
PCOLS = H + 2 * PAD + 2        # 150 padded cols (w-branch, widened)
F32 = mybir.dt.float32
FP16 = mybir.dt.float16
AF = mybir.ActivationFunctionType
ALU = mybir.AluOpType

N_CORES = 8
CH = 512
L_SET = (0, 1, 4, 5, 8, 9)     # lower tap-pair set; uppers are L+2
WB = 3                          # w-branch row-block height
WWIN = 132                      # w-branch widened window


def _kernel_body(tc, a, reps=1):
    for _rep in range(reps):
        _one_pass(tc, a)


def _conv_h(tc, nc, wf, pad, sA, psp):
    """h-branch conv: 4-row chunks ci=0..32, cross-chunk M-pairing."""
    cw = wf[:, 0:704]
    pbuf = {}
    done = {}

    def emit_group(cis):
        for g in L_SET:
            blk = (cw[:, 640:704] if g == 9 else
                   cw[:, (L_SET.index(g)) * 128:(L_SET.index(g)) * 128 + 128])
            for ci in cis:
                if ci == 32 and g == 9:
                    continue
                if g == 0:
                    pbuf[ci] = psp.tile([128, CH], F32, tag="cps", name=f"cph{ci}")
                ps = pbuf[ci][:] if g != 9 else pbuf[ci][0:64, :]
                rhs = pad[:, (4 * ci + 2 * g) * W:(4 * ci + 2 * g) * W + CH]
                nc.tensor.matmul(ps, blk, rhs, start=(g == 0),
                                 stop=(g == 9 or (ci == 32 and g == 8)))
        for ci in cis:
            done[ci] = True
            # evac lower half of ci (chunk ci), upper half of ci (chunk ci-1)
            if ci < 32:
                nc.vector.tensor_copy(sA[0:64, ci * CH:(ci + 1) * CH],
                                      pbuf[ci][0:64, :])
            if ci > 0:
                nc.vector.tensor_copy(sA[64:128, (ci - 1) * CH:ci * CH],
                                      pbuf[ci][64:128, :])

    for g0 in range(0, 32, 4):
        emit_group(range(g0, g0 + 4))
    emit_group([32])


def _conv_w(tc, nc, wf, prr, sA, psp):
    """w-branch conv: 3-row blocks, widened-window M-pairing."""
    cw = wf[:, 704:1408]
    nblk = (H + WB - 1) // WB  # 43 (last block 2 rows)
    for b0 in range(0, nblk, 4):
        blks = range(b0, min(b0 + 4, nblk))
        pbuf = {}
        rows = {}
        for g in L_SET:
            blk = (cw[:, 640:704] if g == 9 else
                   cw[:, (L_SET.index(g)) * 128:(L_SET.index(g)) * 128 + 128])
            for b in blks:
                r = min(WB, H - b * WB)
                rows[b] = r
                if g == 0:
                    pbuf[b] = psp.tile([128, CH], F32, tag="cps",
                                       name=f"cpw{b}")
                ps = (pbuf[b][:, 0:r * WWIN] if g != 9 else
                      pbuf[b][0:64, 0:r * WWIN])
                rhs = prr[:, b * WB:b * WB + r, 2 * g:2 * g + WWIN]
                nc.tensor.matmul(ps, blk, rhs, start=(g == 0), stop=(g == 9))
        for b in blks:
            r = rows[b]
            pr = pbuf[b][:, 0:r * WWIN].rearrange("p (r c) -> p r c", c=WWIN)
            dst = sA[:, b * WB * W:(b * WB + r) * W].rearrange(
                "p (r c) -> p r c", c=W)
            nc.vector.tensor_copy(dst[0:64], pr[0:64, :, 0:W])
            nc.vector.tensor_copy(dst[64:128], pr[64:128, :, 4:4 + W])


def _one_pass(tc, a):
    nc = tc.nc

    # ---------------- pools (alloc order = SBUF layout; LIFO release) ----
    dp = tc.alloc_tile_pool(name="dram", bufs=1, space="DRAM")
    wp = tc.alloc_tile_pool(name="wts", bufs=1)
    xcp = tc.alloc_tile_pool(name="xc", bufs=2)
    evp = tc.alloc_tile_pool(name="evac", bufs=2)
    sprm = tc.alloc_tile_pool(name="sprm", bufs=4)
    sprk = tc.alloc_tile_pool(name="sprk", bufs=3)
    gsb = tc.alloc_tile_pool(name="gsb", bufs=1)
    btev = tc.alloc_tile_pool(name="btev", bufs=3)
    scp = tc.alloc_tile_pool(name="sc", bufs=1)
    stp = tc.alloc_tile_pool(name="stage", bufs=2)
    padp = tc.alloc_tile_pool(name="pads", bufs=1)

    wf = wp.tile([128, 1856], FP16, tag="wf", name="wf")
    nc.sync.dma_start(wf[:], a["wf"])
    wa = wp.tile([128, 9], F32, tag="wa", name="wa")
    nc.sync.dma_start(wa[:], a["wa"])
    qkv1w = [wf[:, 1408:1536], wf[:, 1536:1664]]
    qkv2w = [wf[:, 1664:1728], wf[:, 1728:1792]]
    projw = wf[:, 1792:1856]
    qkv1b = [wa[:, 0:1], wa[:, 1:2]]
    qkv2b = [wa[0:64, 6:7], wa[0:64, 7:8]]
    projb = wa[0:64, 8:9]

    # DRAM parking, all [pixel-row, (chan, pixel-col)] fp16 layouts
    cp_qv = [dp.tile([128, HW], FP16, tag=f"cp_qv{i}", name=f"cp_qv{i}")
             for i in range(2)]
    cp_k = [dp.tile([128, NH * D * H], FP16, tag=f"cp_k{i}", name=f"cp_k{i}")
            for i in range(2)]
    cp_s = dp.tile([128, HW], FP16, tag="cp_s", name="cp_s")

    sA = [scp.tile([128, HW], FP16, tag=f"sA{br}", name=f"sA{br}")
          for br in range(2)]


    # ---------------- phase 1: both BNs together, then conv/qkv ---------
    pad0 = padp.tile([128, PROWS * W], FP16, tag="pad0", name="pad0")
    nc.vector.memset(pad0[0:64, 0:PAD * W], 0.0)
    nc.vector.memset(pad0[0:64, (H + PAD) * W:], 0.0)
    nc.vector.memset(pad0[64:128, 0:(PAD - 1) * W], 0.0)
    nc.vector.memset(pad0[64:128, (H + PAD - 1) * W:], 0.0)
    pad1 = padp.tile([128, H * PCOLS], FP16, tag="pad1", name="pad1")
    prr = pad1[:].rearrange("p (h j) -> p h j", j=PCOLS)
    nc.vector.memset(prr[0:64, :, 0:PAD], 0.0)
    nc.vector.memset(prr[0:64, :, H + PAD:], 0.0)
    nc.vector.memset(prr[64:128, :, 0:PAD - 1], 0.0)
    nc.vector.memset(prr[64:128, :, H + PAD - 1:], 0.0)

    # two x passes: pad0 (h-branch) fills first so conv-h starts early,
    # pad1's slower strided fills stream during conv-h
    for br in range(2):
        av = wa[0:64, 2 + 2 * br:3 + 2 * br]
        bv = wa[0:64, 3 + 2 * br:4 + 2 * br]
        for i in range(8):
            xc = xcp.tile([64, 2048], F32, tag="xc")
            nc.sync.dma_start(xc[:], a["x"][:, i * 2048:(i + 1) * 2048])
            st = stp.tile([64, 2048], FP16, tag="bnst")
            nc.scalar.activation(st[:], xc[:], AF.Identity, bias=bv, scale=av)
            if br == 0:
                nc.sync.dma_start(
                    pad0[0:64, (PAD + 16 * i) * W:(PAD + 16 * i) * W + 2048],
                    st[:])
                nc.sync.dma_start(
                    pad0[64:128,
                         (PAD - 1 + 16 * i) * W:(PAD - 1 + 16 * i) * W + 2048],
                    st[:])
            else:
                s3 = st[:].rearrange("p (h w) -> p h w", w=W)
                nc.sync.dma_start(
                    prr[0:64, 16 * i:16 * i + 16, PAD:PAD + W], s3)
                nc.sync.dma_start(
                    prr[64:128, 16 * i:16 * i + 16, PAD - 1:PAD - 1 + W], s3)

    # G matrices (emitted between qkv1 and qkv2 passes via emit_g())
    def emit_g():
        for gi in range(2):
            for nh in range(NH):
                gps = ps_g.tile([128, 128], F32, tag="g")
                if gi == 0 and nh < 2:
                    qa, va = pft[nh]
                elif gi == 0:
                    qa = sprm.tile([128, D * W], FP16, tag="m", name=f"hq{nh}")
                    nc.sync.dma_start(
                        qa[:], cp_qv[0][:, nh * D * W:(nh + 1) * D * W])
                    va = sprm.tile([128, D * W], FP16, tag="m", name=f"wv{nh}")
                    nc.sync.dma_start(
                        va[:],
                        cp_qv[1][:, (64 + nh * D) * W:(64 + (nh + 1) * D) * W])
                else:
                    qa = sprm.tile([128, D * W], FP16, tag="m", name=f"wq{nh}")
                    nc.sync.dma_start(
                        qa[:], cp_qv[1][:, nh * D * W:(nh + 1) * D * W])
                    va = sprm.tile([128, D * W], FP16, tag="m", name=f"hv{nh}")
                    nc.sync.dma_start(
                        va[:],
                        cp_qv[0][:, (64 + nh * D) * W:(64 + (nh + 1) * D) * W])
                for d in range(D):
                    nc.tensor.matmul(gps[:], qa[:, d * W:(d + 1) * W],
                                     va[:, d * W:(d + 1) * W],
                                     start=(d == 0), stop=(d == D - 1))
                nc.scalar.activation(
                    g_sb[:, (gi * NH + nh) * 128:(gi * NH + nh + 1) * 128],
                    gps[:], AF.Copy)

    def qkv1_pass(br):
        for hi in range(8):
            stq = stp.tile([128, 2048], FP16, tag="stq", name=f"stq{br}_{hi}")
            for j in range(4):
                ci = 4 * hi + j
                ps = ps_conv.tile([128, CH], F32, tag="cps", name=f"q1{br}_{hi}_{j}")
                nc.tensor.matmul(ps[:], qkv1w[br],
                                 sA[br][:, ci * CH:(ci + 1) * CH],
                                 start=True, stop=True)
                if hi % 2 == 0:
                    nc.vector.tensor_scalar_add(stq[:, j * CH:(j + 1) * CH],
                                                ps[:], qkv1b[br])
                else:
                    nc.scalar.activation(stq[:, j * CH:(j + 1) * CH], ps[:],
                                         AF.Identity, bias=qkv1b[br])
            dst = cp_qv[br][16 * hi:16 * hi + 16, :].rearrange(
                "h (c w) -> c h w", w=W)
            nc.scalar.dma_start(dst, stq[:].rearrange("c (h w) -> c h w", w=W))

    def qkv2_pass(br):
        sAr = sA[br][:].rearrange("p (h w) -> p w h", w=W)
        for hi in range(8):
            stk = stp.tile([64, 2048], FP16, tag="stk", name=f"stk{br}_{hi}")
            for j in range(4):
                ci = 4 * hi + j
                pst = ps_conv.tile([128, CH], F32, tag="cps", name=f"q2{br}_{hi}_{j}")
                ps = pst[0:64, :]
                nc.tensor.matmul(ps, qkv2w[br],
                                 sAr[:, 4 * ci:4 * ci + 4, :],
                                 start=True, stop=True)
                if hi % 2 == 1:
                    nc.vector.tensor_scalar_add(stk[:, j * CH:(j + 1) * CH],
                                                ps, qkv2b[br])
                else:
                    nc.scalar.activation(stk[:, j * CH:(j + 1) * CH], ps,
                                         AF.Identity, bias=qkv2b[br])
            dst = cp_k[br][16 * hi:16 * hi + 16, :].rearrange(
                "w (c h) -> c w h", h=H)
            nc.scalar.dma_start(dst, stk[:].rearrange("c (w h) -> c w h", h=H))

    g_sb = gsb.tile([128, 16 * 128], FP16, tag="g_sb")
    ps_g = tc.alloc_tile_pool(name="ps_g", bufs=2, space="PSUM")
    ps_conv = tc.alloc_tile_pool(name="ps_conv", bufs=4, space="PSUM")
    ps_bt = tc.alloc_tile_pool(name="ps_bt", bufs=2, space="PSUM")

    _conv_h(tc, nc, wf, pad0, sA[0], ps_conv)
    _conv_w(tc, nc, wf, prr, sA[1], ps_conv)
    qkv1_pass(0)
    qkv2_pass(0)
    qkv1_pass(1)
    pft = {}
    for nh in range(2):
        qa = sprm.tile([128, D * W], FP16, tag="m", name=f"hq{nh}")
        nc.sync.dma_start(qa[:], cp_qv[0][:, nh * D * W:(nh + 1) * D * W])
        va = sprm.tile([128, D * W], FP16, tag="m", name=f"wv{nh}")
        nc.sync.dma_start(
            va[:], cp_qv[1][:, (64 + nh * D) * W:(64 + (nh + 1) * D) * W])
        pft[nh] = (qa, va)
    qkv2_pass(1)

    # ---------------- phase 2: attention ----------------
    def emit_bt(gi):
        # B^T: k loads pipelined one head ahead; gather-stores follow on sync
        ksrc = cp_k[1] if gi == 0 else cp_k[0]
        kts = {}
        def kload(nh):
            ka = sprk.tile([128, D * H], FP16, tag="k", name=f"k{gi}_{nh}")
            nc.sync.dma_start(ka[:], ksrc[:, nh * D * H:(nh + 1) * D * H])
            kts[nh] = ka
        kload(0)
        for nh in range(NH):
            if nh + 1 < NH:
                kload(nh + 1)
            ka = kts.pop(nh)
            gref = g_sb[:, (gi * NH + nh) * 128:(gi * NH + nh + 1) * 128]
            bt = btev.tile([128, D * W], FP16, tag="btv")
            for half in range(2):
                bps = ps_bt.tile([128, CH], F32, tag="bt")
                for j in range(4):
                    d = 4 * half + j
                    nc.tensor.matmul(bps[:, j * 128:(j + 1) * 128],
                                     ka[:, d * H:(d + 1) * H], gref,
                                     start=True, stop=True)
                if half == 0:
                    nc.vector.tensor_copy(bt[:, 0:CH], bps[:])
                else:
                    nc.scalar.activation(bt[:, CH:2 * CH], bps[:], AF.Copy)
            c0 = gi * 64 + nh * D
            nc.sync.dma_start(
                cp_s[c0:c0 + D, :].rearrange("d (h w) -> h d w", w=W),
                bt[:])

    emit_g()
    emit_bt(0)

    padp.release()
    stp.release()
    scp.release()

    # x prefetch for phase 3 (space freed by pad/stage releases)
    xpf = tc.alloc_tile_pool(name="xpf", bufs=8)
    xfs = []
    for hi in range(8):
        xc = xpf.tile([64, 2048], F32, tag="xp", name=f"xf{hi}")
        nc.scalar.dma_start(xc[:], a["x"][:, hi * 2048:(hi + 1) * 2048])
        xfs.append(xc)

    emit_bt(1)
    ps_bt.release()
    ps_conv.release()

    # ---------------- phase 3: projection + sigmoid + x*sig ----------------
    scp2 = tc.alloc_tile_pool(name="scp2", bufs=1)
    s_cp = scp2.tile([128, HW], FP16, tag="s_cp")
    outp = tc.alloc_tile_pool(name="outp", bufs=4)
    sgp = tc.alloc_tile_pool(name="sgp", bufs=4)
    ps_pj = tc.alloc_tile_pool(name="ps_pj", bufs=4, space="PSUM")

    for hi in range(8):
        nc.sync.dma_start(s_cp[:, hi * 2048:(hi + 1) * 2048],
                          cp_s[:, hi * 2048:(hi + 1) * 2048])
    for hi in range(8):
        xc = xfs[hi]
        yst = outp.tile([64, 2048], F32, tag="yst")
        for j in range(4):
            ci = 4 * hi + j
            pps = ps_pj.tile([64, CH], F32, tag="pj")
            nc.tensor.matmul(pps[:], projw, s_cp[:, ci * CH:(ci + 1) * CH],
                             start=True, stop=True)
            sg = sgp.tile([64, CH], F32, tag="sg")
            nc.scalar.activation(sg[:], pps[:], AF.Sigmoid, bias=projb)
            nc.vector.tensor_mul(yst[:, j * CH:(j + 1) * CH], sg[:],
                                 xc[:, j * CH:(j + 1) * CH])
        nc.scalar.dma_start(a["y"][:, hi * 2048:(hi + 1) * 2048], yst[:])

    for p in (ps_pj, sgp, outp, scp2, xpf, ps_g, btev, gsb, sprk, sprm,
              evp, xcp, wp, dp):
        p.release()


def _prep_weights(inputs):
    """Host-side packing: BN affine, paired conv taps, folded qkv biases."""
    inp = {k: np.asarray(v, dtype=np.float64) for k, v in inputs.items()}
    a1 = inp["bn1_g"] / np.sqrt(inp["bn1_v"] + EPS)
    b1 = inp["bn1_b"] - inp["bn1_m"] * a1
    a2 = inp["bn2_g"] / np.sqrt(inp["bn2_v"] + EPS)
    b2 = inp["bn2_b"] - inp["bn2_m"] * a2

    def conv_pack(ws):
        eff = np.zeros((23, C, C))  # taps 0..20 live; 21,22 stay zero
        for j, k in enumerate(KS):
            off = PAD - k // 2
            for i in range(k):
                eff[off + i] += ws[j][:, :, i]
        pk = np.zeros((128, 704))
        for gi, g in enumerate(L_SET[:-1]):  # 0,1,4,5,8 -> M=128 blocks
            c0 = gi * 128
            pk[0:64, c0:c0 + 64] = eff[2 * g].T
            pk[64:128, c0:c0 + 64] = eff[2 * g + 1].T
            pk[0:64, c0 + 64:c0 + 128] = eff[2 * g + 4].T
            pk[64:128, c0 + 64:c0 + 128] = eff[2 * g + 5].T
        pk[0:64, 640:704] = eff[18].T   # g=9 lower-only block
        pk[64:128, 640:704] = eff[19].T
        return pk

    convh = conv_pack([inp[f"sc1_w{j}"][:, :, :, 0] for j in range(3)])
    convw = conv_pack([inp[f"sc2_w{j}"][:, :, 0, :] for j in range(3)])
    bch = inp["sc1_b0"] + inp["sc1_b1"] + inp["sc1_b2"]
    bcw = inp["sc2_b0"] + inp["sc2_b1"] + inp["sc2_b2"]

    scale = D * H ** (-0.5)
    idx = (np.arange(NH)[:, None] * 24 + np.arange(D)[None, :]).ravel()
    idx_q, idx_k, idx_v = idx, idx + 8, idx + 16

    wf = np.zeros((128, 1856))
    wf[:, 0:704] = convh
    wf[:, 704:1408] = convw
    wa = np.zeros((128, 9))
    wa[0:64, 2] = a1; wa[0:64, 3] = b1
    wa[0:64, 4] = a2; wa[0:64, 5] = b2

    for br, (qw, qb, bc) in enumerate(
            [(inp["hqkv_w"], inp["hqkv_b"], bch),
             (inp["wqkv_w"], inp["wqkv_b"], bcw)]):
        bfold = qb + qw @ bc
        Wq, Wk, Wv = qw[idx_q] * scale, qw[idx_k], qw[idx_v]
        bq, bk, bv = bfold[idx_q] * scale, bfold[idx_k], bfold[idx_v]
        q1 = np.concatenate([Wq.T, Wv.T], axis=1)          # [64, 128]
        wf[0:64, 1408 + 128 * br:1536 + 128 * br] = q1
        wf[64:128, 1408 + 128 * br:1536 + 128 * br] = q1   # dup: sums halves
        wf[0:64, 1664 + 64 * br:1728 + 64 * br] = Wk.T
        wf[64:128, 1664 + 64 * br:1728 + 64 * br] = Wk.T
        wa[:, br] = np.concatenate([bq, bv])
        wa[0:64, 6 + br] = bk

    wf[0:64, 1792:1856] = inp["wout_w"].T
    wf[64:128, 1792:1856] = inp["hout_w"].T
    wa[0:64, 8] = inp["wout_b"] + inp["hout_b"]
    return {"wf": wf.astype(np.float16), "wa": wa.astype(np.float32)}


_NC_CACHE = {}
_RUN_OPTS = {"trace": False}
_LAST_RESULT = {}

_W_SHAPES = {"x": [C, HW], "wf": [128, 1856], "wa": [128, 9]}
_W_DTYPES = {"x": F32, "wf": FP16, "wa": F32}


def _build_nc(reps=1):
    key = f"nc{reps}"
    if key in _NC_CACHE:
        return _NC_CACHE[key]
    nc = bacc.Bacc(trn_type="TRN2", target_bir_lowering=False, debug=False)
    a = {}
    for n, s in _W_SHAPES.items():
        a[n] = nc.dram_tensor(n, s, _W_DTYPES[n], kind="ExternalInput").ap()
    a["y"] = nc.dram_tensor("y", [C, HW], F32, kind="ExternalOutput").ap()
    with tile.TileContext(nc) as tc:
        _kernel_body(tc, a, reps=reps)
    nc.compile()
    _NC_CACHE[key] = nc
    return nc


def _in_maps(inputs):
    w = _prep_weights(inputs)
    x = np.ascontiguousarray(np.asarray(inputs["x"], dtype=np.float32))
    maps = []
    for core in range(N_CORES):
        m = {"x": np.ascontiguousarray(x[core].reshape(C, HW))}
        m.update(w)
        maps.append(m)
    return maps


def kernel(**inputs):
    from concourse.bass_utils import run_bass_kernel_spmd

    nc = _build_nc()
    res = run_bass_kernel_spmd(nc, _in_maps(inputs), core_ids=list(range(N_CORES)),
                               trace=_RUN_OPTS["trace"])
    _LAST_RESULT["res"] = res
    out = np.stack([res.results[i]["y"].reshape(C, H, W) for i in range(N_CORES)])
    return out.astype(np.float32)


if __name__ == "__main__":
    nc = _build_nc()
    print("built ok")


# revision 50
# speedup vs baseline: 1.0744x; 1.0744x over previous
"""Trainium2 Bass kernel for nn_MCA_12214886990440 (strip-conv dual-axis attention).

Sharding: data-parallel over batch B=8 across 8 NeuronCores (params replicated).

Per-core math (one batch element, C=64, H=W=128, NH=8, D=8):
  xh = bn1(x); xw = bn2(x)
  sc_h = 21-tap strip conv along H (3 kernel sizes presummed)
  sc_w = 21-tap strip conv along W
  qkv (conv bias folded into qkv bias); attention per head reassociated:
      w_o^T = wk_t @ G_w,  G_w = sum_d hq(d)^T wv(d)   [128x128 Gram]
      h_o^T = hk_t @ G_h,  G_h = sum_d wq(d)^T hv(d)
  y = x * sigmoid(wout@w_o + hout@h_o + b)

All matmuls run in fp16 (fp32 PSUM accumulate). Conv uses dual pairing:
K=128 packs taps (t, t+1) via a row/col-shifted image copy on partitions
64:128; M=128 packs two tap-pairs, either across adjacent 4-row output
chunks (h-branch) or within a widened 132-column window (w-branch). The
resulting partition-split partial sums are combined for free inside the
qkv matmuls by duplicating the qkv weights across K=128.

Layout pivots go through small DRAM tensors with DMA-friendly layouts:
q/v/k/s are parked as [pixel-row, (channel, pixel-col)] so every
attention-side load is a contiguous [128 x 2KB] transfer.
"""
import sys
sys.path.insert(0, "/opt/trn_rl_repo")

import numpy as np

import concourse.bass as bass
import concourse.tile as tile
from concourse import bacc
from concourse import mybir

B, C, H, W, NH, D = 8, 64, 128, 128, 8, 8
KS = [7, 11, 21]
EPS = 1e-5
PAD = 10          # max k//2
HW = H * W        # 16384
PROWS = H + 2 * PAD            # 148 padded rows (h-branch)
PCOLS = H + 2 * PAD + 2        # 150 padded cols (w-branch, widened)
F32 = mybir.dt.float32
FP16 = mybir.dt.float16
AF = mybir.ActivationFunctionType
ALU = mybir.AluOpType

N_CORES = 8
CH = 512
L_SET = (0, 1, 4, 5, 8, 9)     # lower tap-pair set; uppers are L+2
WB = 3                          # w-branch row-block height
WWIN = 132                      # w-branch widened window


def _kernel_body(tc, a, reps=1):
    for _rep in range(reps):
        _one_pass(tc, a)


def _conv_h(tc, nc, wf, pad, sA, psp):
    """h-branch conv: 4-row chunks ci=0..32, cross-chunk M-pairing."""
    cw = wf[:, 0:704]
    pbuf = {}
    done = {}

    def emit_group(cis):
        for g in L_SET:
            blk = (cw[:, 640:704] if g == 9 else
                   cw[:, (L_SET.index(g)) * 128:(L_SET.index(g)) * 128 + 128])
            for ci in cis:
                if ci == 32 and g == 9:
                    continue
                if g == 0:
                    pbuf[ci] = psp.tile([128, CH], F32, tag="cps", name=f"cph{ci}")
                ps = pbuf[ci][:] if g != 9 else pbuf[ci][0:64, :]
                rhs = pad[:, (4 * ci + 2 * g) * W:(4 * ci + 2 * g) * W + CH]
                nc.tensor.matmul(ps, blk, rhs, start=(g == 0),
                                 stop=(g == 9 or (ci == 32 and g == 8)))
        for ci in cis:
            done[ci] = True
            # evac lower half of ci (chunk ci), upper half of ci (chunk ci-1)
            if ci < 32:
                nc.vector.tensor_copy(sA[0:64, ci * CH:(ci + 1) * CH],
                                      pbuf[ci][0:64, :])
            if ci > 0:
                nc.vector.tensor_copy(sA[64:128, (ci - 1) * CH:ci * CH],
                                      pbuf[ci][64:128, :])

    for g0 in range(0, 32, 4):
        emit_group(range(g0, g0 + 4))
    emit_group([32])


def _conv_w(tc, nc, wf, prr, sA, psp):
    """w-branch conv: 3-row blocks, widened-window M-pairing."""
    cw = wf[:, 704:1408]
    nblk = (H + WB - 1) // WB  # 43 (last block 2 rows)
    for b0 in range(0, nblk, 4):
        blks = range(b0, min(b0 + 4, nblk))
        pbuf = {}
        rows = {}
        for g in L_SET:
            blk = (cw[:, 640:704] if g == 9 else
                   cw[:, (L_SET.index(g)) * 128:(L_SET.index(g)) * 128 + 128])
            for b in blks:
                r = min(WB, H - b * WB)
                rows[b] = r
                if g == 0:
                    pbuf[b] = psp.tile([128, CH], F32, tag="cps",
                                       name=f"cpw{b}")
                ps = (pbuf[b][:, 0:r * WWIN] if g != 9 else
                      pbuf[b][0:64, 0:r * WWIN])
                rhs = prr[:, b * WB:b * WB + r, 2 * g:2 * g + WWIN]
                nc.tensor.matmul(ps, blk, rhs, start=(g == 0), stop=(g == 9))
        for b in blks:
            r = rows[b]
            pr = pbuf[b][:, 0:r * WWIN].rearrange("p (r c) -> p r c", c=WWIN)
            dst = sA[:, b * WB * W:(b * WB + r) * W].rearrange(
                "p (r c) -> p r c", c=W)
            nc.vector.tensor_copy(dst[0:64], pr[0:64, :, 0:W])
            nc.vector.tensor_copy(dst[64:128], pr[64:128, :, 4:4 + W])


def _one_pass(tc, a):
    nc = tc.nc

    # ---------------- pools (alloc order = SBUF layout; LIFO release) ----
    dp = tc.alloc_tile_pool(name="dram", bufs=1, space="DRAM")
    wp = tc.alloc_tile_pool(name="wts", bufs=1)
    xcp = tc.alloc_tile_pool(name="xc", bufs=2)
    evp = tc.alloc_tile_pool(name="evac", bufs=2)
    sprm = tc.alloc_tile_pool(name="sprm", bufs=4)
    sprk = tc.alloc_tile_pool(name="sprk", bufs=3)
    gsb = tc.alloc_tile_pool(name="gsb", bufs=1)
    btev = tc.alloc_tile_pool(name="btev", bufs=3)
    scp = tc.alloc_tile_pool(name="sc", bufs=1)
    stp = tc.alloc_tile_pool(name="stage", bufs=2)
    padp = tc.alloc_tile_pool(name="pads", bufs=1)

    wf = wp.tile([128, 1856], FP16, tag="wf", name="wf")
    nc.sync.dma_start(wf[:], a["wf"])
    wa = wp.tile([128, 9], F32, tag="wa", name="wa")
    nc.sync.dma_start(wa[:], a["wa"])
    qkv1w = [wf[:, 1408:1536], wf[:, 1536:1664]]
    qkv2w = [wf[:, 1664:1728], wf[:, 1728:1792]]
    projw = wf[:, 1792:1856]
    qkv1b = [wa[:, 0:1], wa[:, 1:2]]
    qkv2b = [wa[0:64, 6:7], wa[0:64, 7:8]]
    projb = wa[0:64, 8:9]

    # DRAM parking, all [pixel-row, (chan, pixel-col)] fp16 layouts
    cp_qv = [dp.tile([128, HW], FP16, tag=f"cp_qv{i}", name=f"cp_qv{i}")
             for i in range(2)]
    cp_k = [dp.tile([128, NH * D * H], FP16, tag=f"cp_k{i}", name=f"cp_k{i}")
            for i in range(2)]
    cp_s = dp.tile([128, HW], FP16, tag="cp_s", name="cp_s")

    sA = [scp.tile([128, HW], FP16, tag=f"sA{br}", name=f"sA{br}")
          for br in range(2)]


    # ---------------- phase 1: both BNs together, then conv/qkv ---------
    pad0 = padp.tile([128, PROWS * W], FP16, tag="pad0", name="pad0")
    nc.vector.memset(pad0[0:64, 0:PAD * W], 0.0)
    nc.vector.memset(pad0[0:64, (H + PAD) * W:], 0.0)
    nc.vector.memset(pad0[64:128, 0:(PAD - 1) * W], 0.0)
    nc.vector.memset(pad0[64:128, (H + PAD - 1) * W:], 0.0)
    pad1 = padp.tile([128, H * PCOLS], FP16, tag="pad1", name="pad1")
    prr = pad1[:].rearrange("p (h j) -> p h j", j=PCOLS)
    nc.vector.memset(prr[0:64, :, 0:PAD], 0.0)
    nc.vector.memset(prr[0:64, :, H + PAD:], 0.0)
    nc.vector.memset(prr[64:128, :, 0:PAD - 1], 0.0)
    nc.vector.memset(prr[64:128, :, H + PAD - 1:], 0.0)

    # two x passes: pad0 (h-branch) fills first so conv-h starts early,
    # pad1's slower strided fills stream during conv-h
    for br in range(2):
        av = wa[0:64, 2 + 2 * br:3 + 2 * br]
        bv = wa[0:64, 3 + 2 * br:4 + 2 * br]
        for i in range(8):
            xc = xcp.tile([64, 2048], FP16, tag="xc")
            nc.sync.dma_start(xc[:], a["xh"][:, i * 2048:(i + 1) * 2048])
            st = stp.tile([64, 2048], FP16, tag="bnst")
            nc.scalar.activation(st[:], xc[:], AF.Identity, bias=bv, scale=av)
            if br == 0:
                nc.sync.dma_start(
                    pad0[0:64, (PAD + 16 * i) * W:(PAD + 16 * i) * W + 2048],
                    st[:])
                nc.sync.dma_start(
                    pad0[64:128,
                         (PAD - 1 + 16 * i) * W:(PAD - 1 + 16 * i) * W + 2048],
                    st[:])
            else:
                s3 = st[:].rearrange("p (h w) -> p h w", w=W)
                nc.sync.dma_start(
                    prr[0:64, 16 * i:16 * i + 16, PAD:PAD + W], s3)
                nc.sync.dma_start(
                    prr[64:128, 16 * i:16 * i + 16, PAD - 1:PAD - 1 + W], s3)

    # G matrices (emitted between qkv1 and qkv2 passes via emit_g())
    def emit_g():
        for gi in range(2):
            for nh in range(NH):
                gps = ps_g.tile([128, 128], F32, tag="g")
                if gi == 0 and nh < 2:
                    qa, va = pft[nh]
                elif gi == 0:
                    qa = sprm.tile([128, D * W], FP16, tag="m", name=f"hq{nh}")
                    nc.sync.dma_start(
                        qa[:], cp_qv[0][:, nh * D * W:(nh + 1) * D * W])
                    va = sprm.tile([128, D * W], FP16, tag="m", name=f"wv{nh}")
                    nc.sync.dma_start(
                        va[:],
                        cp_qv[1][:, (64 + nh * D) * W:(64 + (nh + 1) * D) * W])
                else:
                    qa = sprm.tile([128, D * W], FP16, tag="m", name=f"wq{nh}")
                    nc.sync.dma_start(
                        qa[:], cp_qv[1][:, nh * D * W:(nh + 1) * D * W])
                    va = sprm.tile([128, D * W], FP16, tag="m", name=f"hv{nh}")
                    nc.sync.dma_start(
                        va[:],
                        cp_qv[0][:, (64 + nh * D) * W:(64 + (nh + 1) * D) * W])
                for d in range(D):
                    nc.tensor.matmul(gps[:], qa[:, d * W:(d + 1) * W],
                                     va[:, d * W:(d + 1) * W],
                                     start=(d == 0), stop=(d == D - 1))
                nc.scalar.activation(
                    g_sb[:, (gi * NH + nh) * 128:(gi * NH + nh + 1) * 128],
                    gps[:], AF.Copy)

    def qkv1_pass(br):
        for hi in range(8):
            stq = stp.tile([128, 2048], FP16, tag="stq", name=f"stq{br}_{hi}")
            for j in range(4):
                ci = 4 * hi + j
                ps = ps_conv.tile([128, CH], F32, tag="cps", name=f"q1{br}_{hi}_{j}")
                nc.tensor.matmul(ps[:], qkv1w[br],
                                 sA[br][:, ci * CH:(ci + 1) * CH],
                                 start=True, stop=True)
                if hi % 2 == 0:
                    nc.vector.tensor_scalar_add(stq[:, j * CH:(j + 1) * CH],
                                                ps[:], qkv1b[br])
                else:
                    nc.scalar.activation(stq[:, j * CH:(j + 1) * CH], ps[:],
                                         AF.Identity, bias=qkv1b[br])
            dst = cp_qv[br][16 * hi:16 * hi + 16, :].rearrange(
                "h (c w) -> c h w", w=W)
            nc.scalar.dma_start(dst, stq[:].rearrange("c (h w) -> c h w", w=W))

    def qkv2_pass(br):
        sAr = sA[br][:].rearrange("p (h w) -> p w h", w=W)
        for hi in range(8):
            stk = stp.tile([64, 2048], FP16, tag="stk", name=f"stk{br}_{hi}")
            for j in range(4):
                ci = 4 * hi + j
                pst = ps_conv.tile([128, CH], F32, tag="cps", name=f"q2{br}_{hi}_{j}")
                ps = pst[0:64, :]
                nc.tensor.matmul(ps, qkv2w[br],
                                 sAr[:, 4 * ci:4 * ci + 4, :],
                                 start=True, stop=True)
                if hi % 2 == 1:
                    nc.vector.tensor_scalar_add(stk[:, j * CH:(j + 1) * CH],
                                                ps, qkv2b[br])
                else:
                    nc.scalar.activation(stk[:, j * CH:(j + 1) * CH], ps,
                                         AF.Identity, bias=qkv2b[br])
            dst = cp_k[br][16 * hi:16 * hi + 16, :].rearrange(
                "w (c h) -> c w h", h=H)
            nc.scalar.dma_start(dst, stk[:].rearrange("c (w h) -> c w h", h=H))

    g_sb = gsb.tile([128, 16 * 128], FP16, tag="g_sb")
    ps_g = tc.alloc_tile_pool(name="ps_g", bufs=2, space="PSUM")
    ps_conv = tc.alloc_tile_pool(name="ps_conv", bufs=4, space="PSUM")
    ps_bt = tc.alloc_tile_pool(name="ps_bt", bufs=2, space="PSUM")

    _conv_h(tc, nc, wf, pad0, sA[0], ps_conv)
    _conv_w(tc, nc, wf, prr, sA[1], ps_conv)
    qkv1_pass(0)
    qkv2_pass(0)
    qkv1_pass(1)
    pft = {}
    for nh in range(2):
        qa = sprm.tile([128, D * W], FP16, tag="m", name=f"hq{nh}")
        nc.sync.dma_start(qa[:], cp_qv[0][:, nh * D * W:(nh + 1) * D * W])
        va = sprm.tile([128, D * W], FP16, tag="m", name=f"wv{nh}")
        nc.sync.dma_start(
            va[:], cp_qv[1][:, (64 + nh * D) * W:(64 + (nh + 1) * D) * W])
        pft[nh] = (qa, va)
    qkv2_pass(1)

    # ---------------- phase 2: attention ----------------
    def emit_bt(gi):
        # B^T: k loads pipelined one head ahead; gather-stores follow on sync
        ksrc = cp_k[1] if gi == 0 else cp_k[0]
        kts = {}
        def kload(nh):
            ka = sprk.tile([128, D * H], FP16, tag="k", name=f"k{gi}_{nh}")
            nc.sync.dma_start(ka[:], ksrc[:, nh * D * H:(nh + 1) * D * H])
            kts[nh] = ka
        kload(0)
        for nh in range(NH):
            if nh + 1 < NH:
                kload(nh + 1)
            ka = kts.pop(nh)
            gref = g_sb[:, (gi * NH + nh) * 128:(gi * NH + nh + 1) * 128]
            bt = btev.tile([128, D * W], FP16, tag="btv")
            for half in range(2):
                bps = ps_bt.tile([128, CH], F32, tag="bt")
                for j in range(4):
                    d = 4 * half + j
                    nc.tensor.matmul(bps[:, j * 128:(j + 1) * 128],
                                     ka[:, d * H:(d + 1) * H], gref,
                                     start=True, stop=True)
                if half == 0:
                    nc.vector.tensor_copy(bt[:, 0:CH], bps[:])
                else:
                    nc.scalar.activation(bt[:, CH:2 * CH], bps[:], AF.Copy)
            c0 = gi * 64 + nh * D
            nc.sync.dma_start(
                cp_s[c0:c0 + D, :].rearrange("d (h w) -> h d w", w=W),
                bt[:])

    emit_g()
    emit_bt(0)
    emit_bt(1)

    padp.release()
    stp.release()
    scp.release()

    # x prefetch for phase 3 (space freed by pad/stage releases)
    xpf = tc.alloc_tile_pool(name="xpf", bufs=8)
    xfs = []
    for hi in range(8):
        xc = xpf.tile([64, 2048], FP16, tag="xp", name=f"xf{hi}")
        nc.sync.dma_start(xc[:], a["xh"][:, hi * 2048:(hi + 1) * 2048])
        xfs.append(xc)
    ps_bt.release()
    ps_conv.release()

    # ---------------- phase 3: projection + sigmoid + x*sig ----------------
    scp2 = tc.alloc_tile_pool(name="scp2", bufs=1)
    s_cp = scp2.tile([128, HW], FP16, tag="s_cp")
    outp = tc.alloc_tile_pool(name="outp", bufs=4)
    sgp = tc.alloc_tile_pool(name="sgp", bufs=4)
    ps_pj = tc.alloc_tile_pool(name="ps_pj", bufs=4, space="PSUM")

    for hi in range(8):
        nc.sync.dma_start(s_cp[:, hi * 2048:(hi + 1) * 2048],
                          cp_s[:, hi * 2048:(hi + 1) * 2048])
    for hi in range(8):
        xc = xfs[hi]
        yst = outp.tile([64, 2048], F32, tag="yst")
        for j in range(4):
            ci = 4 * hi + j
            pps = ps_pj.tile([64, CH], F32, tag="pj")
            nc.tensor.matmul(pps[:], projw, s_cp[:, ci * CH:(ci + 1) * CH],
                             start=True, stop=True)
            sg = sgp.tile([64, CH], F32, tag="sg")
            nc.scalar.activation(sg[:], pps[:], AF.Sigmoid, bias=projb)
            nc.vector.tensor_mul(yst[:, j * CH:(j + 1) * CH], sg[:],
                                 xc[:, j * CH:(j + 1) * CH])
        nc.scalar.dma_start(a["y"][:, hi * 2048:(hi + 1) * 2048], yst[:])

    for p in (ps_pj, sgp, outp, scp2, xpf, ps_g, btev, gsb, sprk, sprm,
              evp, xcp, wp, dp):
        p.release()


def _prep_weights(inputs):
    """Host-side packing: BN affine, paired conv taps, folded qkv biases."""
    inp = {k: np.asarray(v, dtype=np.float64) for k, v in inputs.items()}
    a1 = inp["bn1_g"] / np.sqrt(inp["bn1_v"] + EPS)
    b1 = inp["bn1_b"] - inp["bn1_m"] * a1
    a2 = inp["bn2_g"] / np.sqrt(inp["bn2_v"] + EPS)
    b2 = inp["bn2_b"] - inp["bn2_m"] * a2

    def conv_pack(ws):
        eff = np.zeros((23, C, C))  # taps 0..20 live; 21,22 stay zero
        for j, k in enumerate(KS):
            off = PAD - k // 2
            for i in range(k):
                eff[off + i] += ws[j][:, :, i]
        pk = np.zeros((128, 704))
        for gi, g in enumerate(L_SET[:-1]):  # 0,1,4,5,8 -> M=128 blocks
            c0 = gi * 128
            pk[0:64, c0:c0 + 64] = eff[2 * g].T
            pk[64:128, c0:c0 + 64] = eff[2 * g + 1].T
            pk[0:64, c0 + 64:c0 + 128] = eff[2 * g + 4].T
            pk[64:128, c0 + 64:c0 + 128] = eff[2 * g + 5].T
        pk[0:64, 640:704] = eff[18].T   # g=9 lower-only block
        pk[64:128, 640:704] = eff[19].T
        return pk

    convh = conv_pack([inp[f"sc1_w{j}"][:, :, :, 0] for j in range(3)])
    convw = conv_pack([inp[f"sc2_w{j}"][:, :, 0, :] for j in range(3)])
    bch = inp["sc1_b0"] + inp["sc1_b1"] + inp["sc1_b2"]
    bcw = inp["sc2_b0"] + inp["sc2_b1"] + inp["sc2_b2"]

    scale = D * H ** (-0.5)
    idx = (np.arange(NH)[:, None] * 24 + np.arange(D)[None, :]).ravel()
    idx_q, idx_k, idx_v = idx, idx + 8, idx + 16

    wf = np.zeros((128, 1856))
    wf[:, 0:704] = convh
    wf[:, 704:1408] = convw
    wa = np.zeros((128, 9))
    wa[0:64, 2] = a1; wa[0:64, 3] = b1
    wa[0:64, 4] = a2; wa[0:64, 5] = b2

    for br, (qw, qb, bc) in enumerate(
            [(inp["hqkv_w"], inp["hqkv_b"], bch),
             (inp["wqkv_w"], inp["wqkv_b"], bcw)]):
        bfold = qb + qw @ bc
        Wq, Wk, Wv = qw[idx_q] * scale, qw[idx_k], qw[idx_v]
        bq, bk, bv = bfold[idx_q] * scale, bfold[idx_k], bfold[idx_v]
        q1 = np.concatenate([Wq.T, Wv.T], axis=1)          # [64, 128]
        wf[0:64, 1408 + 128 * br:1536 + 128 * br] = q1
        wf[64:128, 1408 + 128 * br:1536 + 128 * br] = q1   # dup: sums halves
        wf[0:64, 1664 + 64 * br:1728 + 64 * br] = Wk.T
        wf[64:128, 1664 + 64 * br:1728 + 64 * br] = Wk.T
        wa[:, br] = np.concatenate([bq, bv])
        wa[0:64, 6 + br] = bk

    wf[0:64, 1792:1856] = inp["wout_w"].T
    wf[64:128, 1792:1856] = inp["hout_w"].T
    wa[0:64, 8] = inp["wout_b"] + inp["hout_b"]
    return {"wf": wf.astype(np.float16), "wa": wa.astype(np.float32)}


_NC_CACHE = {}
_RUN_OPTS = {"trace": False}
_LAST_RESULT = {}

_W_SHAPES = {"xh": [C, HW], "wf": [128, 1856], "wa": [128, 9]}
_W_DTYPES = {"xh": FP16, "wf": FP16, "wa": F32}


def _build_nc(reps=1):
    key = f"nc{reps}"
    if key in _NC_CACHE:
        return _NC_CACHE[key]
    nc = bacc.Bacc(trn_type="TRN2", target_bir_lowering=False, debug=False)
    a = {}
    for n, s in _W_SHAPES.items():
        a[n] = nc.dram_tensor(n, s, _W_DTYPES[n], kind="ExternalInput").ap()
    a["y"] = nc.dram_tensor("y", [C, HW], F32, kind="ExternalOutput").ap()
    with tile.TileContext(nc) as tc:
        _kernel_body(tc, a, reps=reps)
    nc.compile()
    _NC_CACHE[key] = nc
    return nc


def _in_maps(inputs):
    w = _prep_weights(inputs)
    x = np.ascontiguousarray(np.asarray(inputs["x"], dtype=np.float32))
    maps = []
    for core in range(N_CORES):
        m = {"xh": np.ascontiguousarray(x[core].reshape(C, HW).astype(np.float16))}
        m.update(w)
        maps.append(m)
    return maps


def kernel(**inputs):
    from concourse.bass_utils import run_bass_kernel_spmd

    nc = _build_nc()
    res = run_bass_kernel_spmd(nc, _in_maps(inputs), core_ids=list(range(N_CORES)),
                               trace=_RUN_OPTS["trace"])
    _LAST_RESULT["res"] = res
    out = np.stack([res.results[i]["y"].reshape(C, H, W) for i in range(N_CORES)])
    return out.astype(np.float32)


if __name__ == "__main__":
    nc = _build_nc()
    print("built ok")


# revision 51
# speedup vs baseline: 1.1587x; 1.0784x over previous
"""Trainium2 Bass kernel for nn_MCA_12214886990440 (strip-conv dual-axis attention).

Sharding: data-parallel over batch B=8 across 8 NeuronCores (params replicated).

Per-core math (one batch element, C=64, H=W=128, NH=8, D=8):
  xh = bn1(x); xw = bn2(x)
  sc_h = 21-tap strip conv along H (3 kernel sizes presummed)
  sc_w = 21-tap strip conv along W
  qkv (conv bias folded into qkv bias); attention per head reassociated:
      w_o^T = wk_t @ G_w,  G_w = sum_d hq(d)^T wv(d)   [128x128 Gram]
      h_o^T = hk_t @ G_h,  G_h = sum_d wq(d)^T hv(d)
  y = x * sigmoid(wout@w_o + hout@h_o + b)

All matmuls run in fp16 (fp32 PSUM accumulate). Conv uses dual pairing:
K=128 packs taps (t, t+1) via a row/col-shifted image copy on partitions
64:128; M=128 packs two tap-pairs, either across adjacent 4-row output
chunks (h-branch) or within a widened 132-column window (w-branch). The
resulting partition-split partial sums are combined for free inside the
qkv matmuls by duplicating the qkv weights across K=128.

Layout pivots go through small DRAM tensors with DMA-friendly layouts:
q/v/k/s are parked as [pixel-row, (channel, pixel-col)] so every
attention-side load is a contiguous [128 x 2KB] transfer.
"""
import sys
sys.path.insert(0, "/opt/trn_rl_repo")

import numpy as np

import concourse.bass as bass
import concourse.tile as tile
from concourse import bacc
from concourse import mybir

B, C, H, W, NH, D = 8, 64, 128, 128, 8, 8
KS = [7, 11, 21]
EPS = 1e-5
PAD = 10          # max k//2
HW = H * W        # 16384
PROWS = H + 2 * PAD            # 148 padded rows (h-branch)
PCOLS = H + 2 * PAD + 2        # 150 padded cols (w-branch, widened)
F32 = mybir.dt.float32
FP16 = mybir.dt.float16
AF = mybir.ActivationFunctionType
ALU = mybir.AluOpType

N_CORES = 8
CH = 512
L_SET = (0, 1, 4, 5, 8, 9)     # lower tap-pair set; uppers are L+2
WB = 3                          # w-branch row-block height
WWIN = 132                      # w-branch widened window


def _kernel_body(tc, a, reps=1):
    for _rep in range(reps):
        _one_pass(tc, a)


def _conv_h(tc, nc, wf, pad, sA, psp):
    """h-branch conv: 4-row chunks ci=0..32, cross-chunk M-pairing."""
    cw = wf[:, 0:704]
    pbuf = {}
    done = {}

    def emit_group(cis):
        for g in L_SET:
            blk = (cw[:, 640:704] if g == 9 else
                   cw[:, (L_SET.index(g)) * 128:(L_SET.index(g)) * 128 + 128])
            for ci in cis:
                if ci == 32 and g == 9:
                    continue
                if g == 0:
                    pbuf[ci] = psp.tile([128, CH], F32, tag="cps", name=f"cph{ci}")
                ps = pbuf[ci][:] if g != 9 else pbuf[ci][0:64, :]
                rhs = pad[:, (4 * ci + 2 * g) * W:(4 * ci + 2 * g) * W + CH]
                nc.tensor.matmul(ps, blk, rhs, start=(g == 0),
                                 stop=(g == 9 or (ci == 32 and g == 8)))
        for ci in cis:
            done[ci] = True
            # evac lower half of ci (chunk ci), upper half of ci (chunk ci-1)
            if ci < 32:
                nc.vector.tensor_copy(sA[0:64, ci * CH:(ci + 1) * CH],
                                      pbuf[ci][0:64, :])
            if ci > 0:
                nc.vector.tensor_copy(sA[64:128, (ci - 1) * CH:ci * CH],
                                      pbuf[ci][64:128, :])

    for g0 in range(0, 32, 4):
        emit_group(range(g0, g0 + 4))
    emit_group([32])


def _conv_w(tc, nc, wf, prr, sA, psp):
    """w-branch conv: 3-row blocks, widened-window M-pairing."""
    cw = wf[:, 704:1408]
    nblk = (H + WB - 1) // WB  # 43 (last block 2 rows)
    for b0 in range(0, nblk, 4):
        blks = range(b0, min(b0 + 4, nblk))
        pbuf = {}
        rows = {}
        for g in L_SET:
            blk = (cw[:, 640:704] if g == 9 else
                   cw[:, (L_SET.index(g)) * 128:(L_SET.index(g)) * 128 + 128])
            for b in blks:
                r = min(WB, H - b * WB)
                rows[b] = r
                if g == 0:
                    pbuf[b] = psp.tile([128, CH], F32, tag="cps",
                                       name=f"cpw{b}")
                ps = (pbuf[b][:, 0:r * WWIN] if g != 9 else
                      pbuf[b][0:64, 0:r * WWIN])
                rhs = prr[:, b * WB:b * WB + r, 2 * g:2 * g + WWIN]
                nc.tensor.matmul(ps, blk, rhs, start=(g == 0), stop=(g == 9))
        for b in blks:
            r = rows[b]
            pr = pbuf[b][:, 0:r * WWIN].rearrange("p (r c) -> p r c", c=WWIN)
            dst = sA[:, b * WB * W:(b * WB + r) * W].rearrange(
                "p (r c) -> p r c", c=W)
            nc.vector.tensor_copy(dst[0:64], pr[0:64, :, 0:W])
            nc.vector.tensor_copy(dst[64:128], pr[64:128, :, 4:4 + W])


def _one_pass(tc, a):
    nc = tc.nc

    # ---------------- pools (alloc order = SBUF layout; LIFO release) ----
    dp = tc.alloc_tile_pool(name="dram", bufs=1, space="DRAM")
    wp = tc.alloc_tile_pool(name="wts", bufs=1)
    xcp = tc.alloc_tile_pool(name="xc", bufs=2)
    evp = tc.alloc_tile_pool(name="evac", bufs=2)
    sprm = tc.alloc_tile_pool(name="sprm", bufs=4)
    sprk = tc.alloc_tile_pool(name="sprk", bufs=3)
    gsb = tc.alloc_tile_pool(name="gsb", bufs=1)
    btev = tc.alloc_tile_pool(name="btev", bufs=3)
    scp = tc.alloc_tile_pool(name="sc", bufs=1)
    stp = tc.alloc_tile_pool(name="stage", bufs=2)
    padp = tc.alloc_tile_pool(name="pads", bufs=1)

    wf = wp.tile([128, 1856], FP16, tag="wf", name="wf")
    nc.sync.dma_start(wf[:], a["wf"])
    wa = wp.tile([128, 9], F32, tag="wa", name="wa")
    nc.sync.dma_start(wa[:], a["wa"])
    qkv1w = [wf[:, 1408:1536], wf[:, 1536:1664]]
    qkv2w = [wf[:, 1664:1728], wf[:, 1728:1792]]
    projw = wf[:, 1792:1856]
    qkv1b = [wa[:, 0:1], wa[:, 1:2]]
    qkv2b = [wa[0:64, 6:7], wa[0:64, 7:8]]
    projb = wa[0:64, 8:9]

    # DRAM parking, all [pixel-row, (chan, pixel-col)] fp16 layouts
    cp_qv = [dp.tile([128, HW], FP16, tag=f"cp_qv{i}", name=f"cp_qv{i}")
             for i in range(2)]
    cp_k = [dp.tile([128, NH * D * H], FP16, tag=f"cp_k{i}", name=f"cp_k{i}")
            for i in range(2)]
    cp_s = dp.tile([128, HW], FP16, tag="cp_s", name="cp_s")

    sA = [scp.tile([128, HW], FP16, tag=f"sA{br}", name=f"sA{br}")
          for br in range(2)]


    # ---------------- phase 1: both BNs together, then conv/qkv ---------
    pad0 = padp.tile([128, PROWS * W], FP16, tag="pad0", name="pad0")
    nc.vector.memset(pad0[0:64, 0:PAD * W], 0.0)
    nc.vector.memset(pad0[0:64, (H + PAD) * W:], 0.0)
    nc.vector.memset(pad0[64:128, 0:(PAD - 1) * W], 0.0)
    nc.vector.memset(pad0[64:128, (H + PAD - 1) * W:], 0.0)
    pad1 = padp.tile([128, H * PCOLS], FP16, tag="pad1", name="pad1")
    prr = pad1[:].rearrange("p (h j) -> p h j", j=PCOLS)
    nc.vector.memset(prr[0:64, :, 0:PAD], 0.0)
    nc.vector.memset(prr[0:64, :, H + PAD:], 0.0)
    nc.vector.memset(prr[64:128, :, 0:PAD - 1], 0.0)
    nc.vector.memset(prr[64:128, :, H + PAD - 1:], 0.0)

    # two x passes: pad0 (h-branch) fills first so conv-h starts early,
    # pad1's slower strided fills stream during conv-h
    for br in range(2):
        av = wa[0:64, 2 + 2 * br:3 + 2 * br]
        bv = wa[0:64, 3 + 2 * br:4 + 2 * br]
        for i in range(8):
            xc = xcp.tile([64, 2048], F32, tag="xc")
            nc.sync.dma_start(xc[:], a["x"][:, i * 2048:(i + 1) * 2048])
            st = stp.tile([64, 2048], FP16, tag="bnst")
            nc.scalar.activation(st[:], xc[:], AF.Identity, bias=bv, scale=av)
            if br == 0:
                nc.sync.dma_start(
                    pad0[0:64, (PAD + 16 * i) * W:(PAD + 16 * i) * W + 2048],
                    st[:])
                nc.sync.dma_start(
                    pad0[64:128,
                         (PAD - 1 + 16 * i) * W:(PAD - 1 + 16 * i) * W + 2048],
                    st[:])
            else:
                s3 = st[:].rearrange("p (h w) -> p h w", w=W)
                nc.sync.dma_start(
                    prr[0:64, 16 * i:16 * i + 16, PAD:PAD + W], s3)
                nc.sync.dma_start(
                    prr[64:128, 16 * i:16 * i + 16, PAD - 1:PAD - 1 + W], s3)

    # G matrices (emitted between qkv1 and qkv2 passes via emit_g())
    def emit_g():
        for gi in range(2):
            for nh in range(NH):
                gps = ps_g.tile([128, 128], F32, tag="g")
                if gi == 0 and nh < 2:
                    qa, va = pft[nh]
                elif gi == 0:
                    qa = sprm.tile([128, D * W], FP16, tag="m", name=f"hq{nh}")
                    nc.sync.dma_start(
                        qa[:], cp_qv[0][:, nh * D * W:(nh + 1) * D * W])
                    va = sprm.tile([128, D * W], FP16, tag="m", name=f"wv{nh}")
                    nc.sync.dma_start(
                        va[:],
                        cp_qv[1][:, (64 + nh * D) * W:(64 + (nh + 1) * D) * W])
                else:
                    qa = sprm.tile([128, D * W], FP16, tag="m", name=f"wq{nh}")
                    nc.sync.dma_start(
                        qa[:], cp_qv[1][:, nh * D * W:(nh + 1) * D * W])
                    va = sprm.tile([128, D * W], FP16, tag="m", name=f"hv{nh}")
                    nc.sync.dma_start(
                        va[:],
                        cp_qv[0][:, (64 + nh * D) * W:(64 + (nh + 1) * D) * W])
                for d in range(D):
                    nc.tensor.matmul(gps[:], qa[:, d * W:(d + 1) * W],
                                     va[:, d * W:(d + 1) * W],
                                     start=(d == 0), stop=(d == D - 1))
                nc.scalar.activation(
                    g_sb[:, (gi * NH + nh) * 128:(gi * NH + nh + 1) * 128],
                    gps[:], AF.Copy)

    def qkv1_pass(br):
        for hi in range(8):
            stq = stp.tile([128, 2048], FP16, tag="stq", name=f"stq{br}_{hi}")
            for j in range(4):
                ci = 4 * hi + j
                ps = ps_conv.tile([128, CH], F32, tag="cps", name=f"q1{br}_{hi}_{j}")
                nc.tensor.matmul(ps[:], qkv1w[br],
                                 sA[br][:, ci * CH:(ci + 1) * CH],
                                 start=True, stop=True)
                if hi % 2 == 0:
                    nc.vector.tensor_scalar_add(stq[:, j * CH:(j + 1) * CH],
                                                ps[:], qkv1b[br])
                else:
                    nc.scalar.activation(stq[:, j * CH:(j + 1) * CH], ps[:],
                                         AF.Identity, bias=qkv1b[br])
            dst = cp_qv[br][16 * hi:16 * hi + 16, :].rearrange(
                "h (c w) -> c h w", w=W)
            nc.scalar.dma_start(dst, stq[:].rearrange("c (h w) -> c h w", w=W))

    def qkv2_pass(br):
        sAr = sA[br][:].rearrange("p (h w) -> p w h", w=W)
        for hi in range(8):
            stk = stp.tile([64, 2048], FP16, tag="stk", name=f"stk{br}_{hi}")
            for j in range(4):
                ci = 4 * hi + j
                pst = ps_conv.tile([128, CH], F32, tag="cps", name=f"q2{br}_{hi}_{j}")
                ps = pst[0:64, :]
                nc.tensor.matmul(ps, qkv2w[br],
                                 sAr[:, 4 * ci:4 * ci + 4, :],
                                 start=True, stop=True)
                if hi % 2 == 1:
                    nc.vector.tensor_scalar_add(stk[:, j * CH:(j + 1) * CH],
                                                ps, qkv2b[br])
                else:
                    nc.scalar.activation(stk[:, j * CH:(j + 1) * CH], ps,
                                         AF.Identity, bias=qkv2b[br])
            dst = cp_k[br][16 * hi:16 * hi + 16, :].rearrange(
                "w (c h) -> c w h", h=H)
            nc.scalar.dma_start(dst, stk[:].rearrange("c (w h) -> c w h", h=H))

    g_sb = gsb.tile([128, 16 * 128], FP16, tag="g_sb")
    ps_g = tc.alloc_tile_pool(name="ps_g", bufs=2, space="PSUM")
    ps_conv = tc.alloc_tile_pool(name="ps_conv", bufs=4, space="PSUM")
    ps_bt = tc.alloc_tile_pool(name="ps_bt", bufs=2, space="PSUM")

    _conv_h(tc, nc, wf, pad0, sA[0], ps_conv)
    _conv_w(tc, nc, wf, prr, sA[1], ps_conv)
    qkv1_pass(0)
    qkv2_pass(0)
    qkv1_pass(1)
    pft = {}
    for nh in range(2):
        qa = sprm.tile([128, D * W], FP16, tag="m", name=f"hq{nh}")
        nc.sync.dma_start(qa[:], cp_qv[0][:, nh * D * W:(nh + 1) * D * W])
        va = sprm.tile([128, D * W], FP16, tag="m", name=f"wv{nh}")
        nc.sync.dma_start(
            va[:], cp_qv[1][:, (64 + nh * D) * W:(64 + (nh + 1) * D) * W])
        pft[nh] = (qa, va)
    qkv2_pass(1)

    # ---------------- phase 2: attention ----------------
    def emit_bt(gi):
        # B^T: k loads pipelined one head ahead; gather-stores follow on sync
        ksrc = cp_k[1] if gi == 0 else cp_k[0]
        kts = {}
        def kload(nh):
            ka = sprk.tile([128, D * H], FP16, tag="k", name=f"k{gi}_{nh}")
            nc.sync.dma_start(ka[:], ksrc[:, nh * D * H:(nh + 1) * D * H])
            kts[nh] = ka
        kload(0)
        for nh in range(NH):
            if nh + 1 < NH:
                kload(nh + 1)
            ka = kts.pop(nh)
            gref = g_sb[:, (gi * NH + nh) * 128:(gi * NH + nh + 1) * 128]
            bt = btev.tile([128, D * W], FP16, tag="btv")
            for half in range(2):
                bps = ps_bt.tile([128, CH], F32, tag="bt")
                for j in range(4):
                    d = 4 * half + j
                    nc.tensor.matmul(bps[:, j * 128:(j + 1) * 128],
                                     ka[:, d * H:(d + 1) * H], gref,
                                     start=True, stop=True)
                if half == 0:
                    nc.vector.tensor_copy(bt[:, 0:CH], bps[:])
                else:
                    nc.scalar.activation(bt[:, CH:2 * CH], bps[:], AF.Copy)
            c0 = gi * 64 + nh * D
            nc.sync.dma_start(
                cp_s[c0:c0 + D, :].rearrange("d (h w) -> h d w", w=W),
                bt[:])

    emit_g()
    emit_bt(0)
    emit_bt(1)

    padp.release()
    stp.release()
    scp.release()

    # x prefetch for phase 3 (space freed by pad/stage releases)
    xpf = tc.alloc_tile_pool(name="xpf", bufs=8)
    xfs = []
    for hi in range(8):
        xc = xpf.tile([64, 2048], F32, tag="xp", name=f"xf{hi}")
        nc.sync.dma_start(xc[:], a["x"][:, hi * 2048:(hi + 1) * 2048])
        xfs.append(xc)
    ps_bt.release()
    ps_conv.release()

    # ---------------- phase 3: projection + sigmoid + x*sig ----------------
    scp2 = tc.alloc_tile_pool(name="scp2", bufs=1)
    s_cp = scp2.tile([128, HW], FP16, tag="s_cp")
    outp = tc.alloc_tile_pool(name="outp", bufs=4)
    sgp = tc.alloc_tile_pool(name="sgp", bufs=4)
    ps_pj = tc.alloc_tile_pool(name="ps_pj", bufs=4, space="PSUM")

    for hi in range(8):
        nc.sync.dma_start(s_cp[:, hi * 2048:(hi + 1) * 2048],
                          cp_s[:, hi * 2048:(hi + 1) * 2048])
    for hi in range(8):
        xc = xfs[hi]
        yst = outp.tile([64, 2048], F32, tag="yst")
        for j in range(4):
            ci = 4 * hi + j
            pps = ps_pj.tile([64, CH], F32, tag="pj")
            nc.tensor.matmul(pps[:], projw, s_cp[:, ci * CH:(ci + 1) * CH],
                             start=True, stop=True)
            sg = sgp.tile([64, CH], F32, tag="sg")
            nc.scalar.activation(sg[:], pps[:], AF.Sigmoid, bias=projb)
            nc.vector.tensor_mul(yst[:, j * CH:(j + 1) * CH], sg[:],
                                 xc[:, j * CH:(j + 1) * CH])
        nc.scalar.dma_start(a["y"][:, hi * 2048:(hi + 1) * 2048], yst[:])

    for p in (ps_pj, sgp, outp, scp2, xpf, ps_g, btev, gsb, sprk, sprm,
              evp, xcp, wp, dp):
        p.release()


def _prep_weights(inputs):
    """Host-side packing: BN affine, paired conv taps, folded qkv biases."""
    inp = {k: np.asarray(v, dtype=np.float64) for k, v in inputs.items()}
    a1 = inp["bn1_g"] / np.sqrt(inp["bn1_v"] + EPS)
    b1 = inp["bn1_b"] - inp["bn1_m"] * a1
    a2 = inp["bn2_g"] / np.sqrt(inp["bn2_v"] + EPS)
    b2 = inp["bn2_b"] - inp["bn2_m"] * a2

    def conv_pack(ws):
        eff = np.zeros((23, C, C))  # taps 0..20 live; 21,22 stay zero
        for j, k in enumerate(KS):
            off = PAD - k // 2
            for i in range(k):
                eff[off + i] += ws[j][:, :, i]
        pk = np.zeros((128, 704))
        for gi, g in enumerate(L_SET[:-1]):  # 0,1,4,5,8 -> M=128 blocks
            c0 = gi * 128
            pk[0:64, c0:c0 + 64] = eff[2 * g].T
            pk[64:128, c0:c0 + 64] = eff[2 * g + 1].T
            pk[0:64, c0 + 64:c0 + 128] = eff[2 * g + 4].T
            pk[64:128, c0 + 64:c0 + 128] = eff[2 * g + 5].T
        pk[0:64, 640:704] = eff[18].T   # g=9 lower-only block
        pk[64:128, 640:704] = eff[19].T
        return pk

    convh = conv_pack([inp[f"sc1_w{j}"][:, :, :, 0] for j in range(3)])
    convw = conv_pack([inp[f"sc2_w{j}"][:, :, 0, :] for j in range(3)])
    bch = inp["sc1_b0"] + inp["sc1_b1"] + inp["sc1_b2"]
    bcw = inp["sc2_b0"] + inp["sc2_b1"] + inp["sc2_b2"]

    scale = D * H ** (-0.5)
    idx = (np.arange(NH)[:, None] * 24 + np.arange(D)[None, :]).ravel()
    idx_q, idx_k, idx_v = idx, idx + 8, idx + 16

    wf = np.zeros((128, 1856))
    wf[:, 0:704] = convh
    wf[:, 704:1408] = convw
    wa = np.zeros((128, 9))
    wa[0:64, 2] = a1; wa[0:64, 3] = b1
    wa[0:64, 4] = a2; wa[0:64, 5] = b2

    for br, (qw, qb, bc) in enumerate(
            [(inp["hqkv_w"], inp["hqkv_b"], bch),
             (inp["wqkv_w"], inp["wqkv_b"], bcw)]):
        bfold = qb + qw @ bc
        Wq, Wk, Wv = qw[idx_q] * scale, qw[idx_k], qw[idx_v]
        bq, bk, bv = bfold[idx_q] * scale, bfold[idx_k], bfold[idx_v]
        q1 = np.concatenate([Wq.T, Wv.T], axis=1)          # [64, 128]
        wf[0:64, 1408 + 128 * br:1536 + 128 * br] = q1
        wf[64:128, 1408 + 128 * br:1536 + 128 * br] = q1   # dup: sums halves
        wf[0:64, 1664 + 64 * br:1728 + 64 * br] = Wk.T
        wf[64:128, 1664 + 64 * br:1728 + 64 * br] = Wk.T
        wa[:, br] = np.concatenate([bq, bv])
        wa[0:64, 6 + br] = bk

    wf[0:64, 1792:1856] = inp["wout_w"].T
    wf[64:128, 1792:1856] = inp["hout_w"].T
    wa[0:64, 8] = inp["wout_b"] + inp["hout_b"]
    return {"wf": wf.astype(np.float16), "wa": wa.astype(np.float32)}


_NC_CACHE = {}
_RUN_OPTS = {"trace": False}
_LAST_RESULT = {}

_W_SHAPES = {"x": [C, HW], "wf": [128, 1856], "wa": [128, 9]}
_W_DTYPES = {"x": F32, "wf": FP16, "wa": F32}


def _build_nc(reps=1):
    key = f"nc{reps}"
    if key in _NC_CACHE:
        return _NC_CACHE[key]
    nc = bacc.Bacc(trn_type="TRN2", target_bir_lowering=False, debug=False)
    a = {}
    for n, s in _W_SHAPES.items():
        a[n] = nc.dram_tensor(n, s, _W_DTYPES[n], kind="ExternalInput").ap()
    a["y"] = nc.dram_tensor("y", [C, HW], F32, kind="ExternalOutput").ap()
    with tile.TileContext(nc) as tc:
        _kernel_body(tc, a, reps=reps)
    nc.compile()
    _NC_CACHE[key] = nc
    return nc


def _in_maps(inputs):
    w = _prep_weights(inputs)
    x = np.ascontiguousarray(np.asarray(inputs["x"], dtype=np.float32))
    maps = []
    for core in range(N_CORES):
        m = {"x": np.ascontiguousarray(x[core].reshape(C, HW))}
        m.update(w)
        maps.append(m)
    return maps


def kernel(**inputs):
    from concourse.bass_utils import run_bass_kernel_spmd

    nc = _build_nc()
    res = run_bass_kernel_spmd(nc, _in_maps(inputs), core_ids=list(range(N_CORES)),
                               trace=_RUN_OPTS["trace"])
    _LAST_RESULT["res"] = res
    out = np.stack([res.results[i]["y"].reshape(C, H, W) for i in range(N_CORES)])
    return out.astype(np.float32)


if __name__ == "__main__":
    nc = _build_nc()
    print("built ok")


# revision 52
# speedup vs baseline: 1.1943x; 1.0308x over previous
"""Trainium2 Bass kernel for nn_MCA_12214886990440 (strip-conv dual-axis attention).

Sharding: data-parallel over batch B=8 across 8 NeuronCores (params replicated).

Per-core math (one batch element, C=64, H=W=128, NH=8, D=8):
  xh = bn1(x); xw = bn2(x)
  sc_h = 21-tap strip conv along H (3 kernel sizes presummed)
  sc_w = 21-tap strip conv along W
  qkv (conv bias folded into qkv bias); attention per head reassociated:
      w_o^T = wk_t @ G_w,  G_w = sum_d hq(d)^T wv(d)   [128x128 Gram]
      h_o^T = hk_t @ G_h,  G_h = sum_d wq(d)^T hv(d)
  y = x * sigmoid(wout@w_o + hout@h_o + b)

All matmuls run in fp16 (fp32 PSUM accumulate). Conv uses dual pairing:
K=128 packs taps (t, t+1) via a row/col-shifted image copy on partitions
64:128; M=128 packs two tap-pairs, either across adjacent 4-row output
chunks (h-branch) or within a widened 132-column window (w-branch). The
resulting partition-split partial sums are combined for free inside the
qkv matmuls by duplicating the qkv weights across K=128.

Layout pivots go through small DRAM tensors with DMA-friendly layouts:
q/v/k/s are parked as [pixel-row, (channel, pixel-col)] so every
attention-side load is a contiguous [128 x 2KB] transfer.
"""
import sys
sys.path.insert(0, "/opt/trn_rl_repo")

import numpy as np

import concourse.bass as bass
import concourse.tile as tile
from concourse import bacc
from concourse import mybir

B, C, H, W, NH, D = 8, 64, 128, 128, 8, 8
KS = [7, 11, 21]
EPS = 1e-5
PAD = 10          # max k//2
HW = H * W        # 16384
PROWS = H + 2 * PAD            # 148 padded rows (h-branch)
PCOLS = H + 2 * PAD + 2        # 150 padded cols (w-branch, widened)
F32 = mybir.dt.float32
FP16 = mybir.dt.float16
AF = mybir.ActivationFunctionType
ALU = mybir.AluOpType

N_CORES = 8
CH = 512
L_SET = (0, 1, 4, 5, 8, 9)     # lower tap-pair set; uppers are L+2
WB = 3                          # w-branch row-block height
WWIN = 132                      # w-branch widened window


def _kernel_body(tc, a, reps=1):
    for _rep in range(reps):
        _one_pass(tc, a)


def _conv_h(tc, nc, wf, pad, sA, psp):
    """h-branch conv: 4-row chunks ci=0..32, cross-chunk M-pairing."""
    cw = wf[:, 0:704]
    pbuf = {}
    done = {}

    def emit_group(cis):
        for g in L_SET:
            blk = (cw[:, 640:704] if g == 9 else
                   cw[:, (L_SET.index(g)) * 128:(L_SET.index(g)) * 128 + 128])
            for ci in cis:
                if ci == 32 and g == 9:
                    continue
                if g == 0:
                    pbuf[ci] = psp.tile([128, CH], F32, tag="cps", name=f"cph{ci}")
                ps = pbuf[ci][:] if g != 9 else pbuf[ci][0:64, :]
                rhs = pad[:, (4 * ci + 2 * g) * W:(4 * ci + 2 * g) * W + CH]
                nc.tensor.matmul(ps, blk, rhs, start=(g == 0),
                                 stop=(g == 9 or (ci == 32 and g == 8)))
        for ci in cis:
            done[ci] = True
            # evac lower half of ci (chunk ci), upper half of ci (chunk ci-1)
            if ci < 32:
                nc.vector.tensor_copy(sA[0:64, ci * CH:(ci + 1) * CH],
                                      pbuf[ci][0:64, :])
            if ci > 0:
                nc.vector.tensor_copy(sA[64:128, (ci - 1) * CH:ci * CH],
                                      pbuf[ci][64:128, :])

    for g0 in range(0, 32, 4):
        emit_group(range(g0, g0 + 4))
    emit_group([32])


def _conv_w(tc, nc, wf, prr, sA, psp):
    """w-branch conv: 3-row blocks, widened-window M-pairing."""
    cw = wf[:, 704:1408]
    nblk = (H + WB - 1) // WB  # 43 (last block 2 rows)
    for b0 in range(0, nblk, 4):
        blks = range(b0, min(b0 + 4, nblk))
        pbuf = {}
        rows = {}
        for g in L_SET:
            blk = (cw[:, 640:704] if g == 9 else
                   cw[:, (L_SET.index(g)) * 128:(L_SET.index(g)) * 128 + 128])
            for b in blks:
                r = min(WB, H - b * WB)
                rows[b] = r
                if g == 0:
                    pbuf[b] = psp.tile([128, CH], F32, tag="cps",
                                       name=f"cpw{b}")
                ps = (pbuf[b][:, 0:r * WWIN] if g != 9 else
                      pbuf[b][0:64, 0:r * WWIN])
                rhs = prr[:, b * WB:b * WB + r, 2 * g:2 * g + WWIN]
                nc.tensor.matmul(ps, blk, rhs, start=(g == 0), stop=(g == 9))
        for b in blks:
            r = rows[b]
            pr = pbuf[b][:, 0:r * WWIN].rearrange("p (r c) -> p r c", c=WWIN)
            dst = sA[:, b * WB * W:(b * WB + r) * W].rearrange(
                "p (r c) -> p r c", c=W)
            nc.vector.tensor_copy(dst[0:64], pr[0:64, :, 0:W])
            nc.vector.tensor_copy(dst[64:128], pr[64:128, :, 4:4 + W])


def _one_pass(tc, a):
    nc = tc.nc

    # ---------------- pools (alloc order = SBUF layout; LIFO release) ----
    dp = tc.alloc_tile_pool(name="dram", bufs=1, space="DRAM")
    wp = tc.alloc_tile_pool(name="wts", bufs=1)
    xcp = tc.alloc_tile_pool(name="xc", bufs=2)
    evp = tc.alloc_tile_pool(name="evac", bufs=2)
    sprm = tc.alloc_tile_pool(name="sprm", bufs=4)
    sprk = tc.alloc_tile_pool(name="sprk", bufs=3)
    gsb = tc.alloc_tile_pool(name="gsb", bufs=1)
    btev = tc.alloc_tile_pool(name="btev", bufs=3)
    scp = tc.alloc_tile_pool(name="sc", bufs=1)
    stp = tc.alloc_tile_pool(name="stage", bufs=2)
    padp = tc.alloc_tile_pool(name="pads", bufs=1)

    wf = wp.tile([128, 1856], FP16, tag="wf", name="wf")
    nc.sync.dma_start(wf[:], a["wf"])
    wa = wp.tile([128, 9], F32, tag="wa", name="wa")
    nc.sync.dma_start(wa[:], a["wa"])
    qkv1w = [wf[:, 1408:1536], wf[:, 1536:1664]]
    qkv2w = [wf[:, 1664:1728], wf[:, 1728:1792]]
    projw = wf[:, 1792:1856]
    qkv1b = [wa[:, 0:1], wa[:, 1:2]]
    qkv2b = [wa[0:64, 6:7], wa[0:64, 7:8]]
    projb = wa[0:64, 8:9]

    # DRAM parking, all [pixel-row, (chan, pixel-col)] fp16 layouts
    cp_qv = [dp.tile([128, HW], FP16, tag=f"cp_qv{i}", name=f"cp_qv{i}")
             for i in range(2)]
    cp_k = [dp.tile([128, NH * D * H], FP16, tag=f"cp_k{i}", name=f"cp_k{i}")
            for i in range(2)]
    cp_s = dp.tile([128, HW], FP16, tag="cp_s", name="cp_s")

    sA = [scp.tile([128, HW], FP16, tag=f"sA{br}", name=f"sA{br}")
          for br in range(2)]


    # ---------------- phase 1: both BNs together, then conv/qkv ---------
    pad0 = padp.tile([128, PROWS * W], FP16, tag="pad0", name="pad0")
    nc.vector.memset(pad0[0:64, 0:PAD * W], 0.0)
    nc.vector.memset(pad0[0:64, (H + PAD) * W:], 0.0)
    nc.vector.memset(pad0[64:128, 0:(PAD - 1) * W], 0.0)
    nc.vector.memset(pad0[64:128, (H + PAD - 1) * W:], 0.0)
    pad1 = padp.tile([128, H * PCOLS], FP16, tag="pad1", name="pad1")
    prr = pad1[:].rearrange("p (h j) -> p h j", j=PCOLS)
    nc.vector.memset(prr[0:64, :, 0:PAD], 0.0)
    nc.vector.memset(prr[0:64, :, H + PAD:], 0.0)
    nc.vector.memset(prr[64:128, :, 0:PAD - 1], 0.0)
    nc.vector.memset(prr[64:128, :, H + PAD - 1:], 0.0)

    # two x passes: pad0 (h-branch) fills first so conv-h starts early,
    # pad1's slower strided fills stream during conv-h
    for br in range(2):
        av = wa[0:64, 2 + 2 * br:3 + 2 * br]
        bv = wa[0:64, 3 + 2 * br:4 + 2 * br]
        for i in range(8):
            xc = xcp.tile([64, 2048], F32, tag="xc")
            nc.sync.dma_start(xc[:], a["x"][:, i * 2048:(i + 1) * 2048])
            st = stp.tile([64, 2048], FP16, tag="bnst")
            nc.scalar.activation(st[:], xc[:], AF.Identity, bias=bv, scale=av)
            if br == 0:
                nc.sync.dma_start(
                    pad0[0:64, (PAD + 16 * i) * W:(PAD + 16 * i) * W + 2048],
                    st[:])
                nc.sync.dma_start(
                    pad0[64:128,
                         (PAD - 1 + 16 * i) * W:(PAD - 1 + 16 * i) * W + 2048],
                    st[:])
            else:
                s3 = st[:].rearrange("p (h w) -> p h w", w=W)
                nc.sync.dma_start(
                    prr[0:64, 16 * i:16 * i + 16, PAD:PAD + W], s3)
                nc.sync.dma_start(
                    prr[64:128, 16 * i:16 * i + 16, PAD - 1:PAD - 1 + W], s3)

    # G matrices (emitted between qkv1 and qkv2 passes via emit_g())
    def emit_g():
        for gi in range(2):
            for nh in range(NH):
                gps = ps_g.tile([128, 128], F32, tag="g")
                if gi == 0 and nh < 2:
                    qa, va = pft[nh]
                elif gi == 0:
                    qa = sprm.tile([128, D * W], FP16, tag="m", name=f"hq{nh}")
                    nc.sync.dma_start(
                        qa[:], cp_qv[0][:, nh * D * W:(nh + 1) * D * W])
                    va = sprm.tile([128, D * W], FP16, tag="m", name=f"wv{nh}")
                    nc.sync.dma_start(
                        va[:],
                        cp_qv[1][:, (64 + nh * D) * W:(64 + (nh + 1) * D) * W])
                else:
                    qa = sprm.tile([128, D * W], FP16, tag="m", name=f"wq{nh}")
                    nc.sync.dma_start(
                        qa[:], cp_qv[1][:, nh * D * W:(nh + 1) * D * W])
                    va = sprm.tile([128, D * W], FP16, tag="m", name=f"hv{nh}")
                    nc.sync.dma_start(
                        va[:],
                        cp_qv[0][:, (64 + nh * D) * W:(64 + (nh + 1) * D) * W])
                for d in range(D):
                    nc.tensor.matmul(gps[:], qa[:, d * W:(d + 1) * W],
                                     va[:, d * W:(d + 1) * W],
                                     start=(d == 0), stop=(d == D - 1))
                nc.scalar.activation(
                    g_sb[:, (gi * NH + nh) * 128:(gi * NH + nh + 1) * 128],
                    gps[:], AF.Copy)

    def qkv1_pass(br):
        for hi in range(8):
            stq = stp.tile([128, 2048], FP16, tag="stq", name=f"stq{br}_{hi}")
            for j in range(4):
                ci = 4 * hi + j
                ps = ps_conv.tile([128, CH], F32, tag="cps", name=f"q1{br}_{hi}_{j}")
                nc.tensor.matmul(ps[:], qkv1w[br],
                                 sA[br][:, ci * CH:(ci + 1) * CH],
                                 start=True, stop=True)
                if hi % 2 == 0:
                    nc.vector.tensor_scalar_add(stq[:, j * CH:(j + 1) * CH],
                                                ps[:], qkv1b[br])
                else:
                    nc.scalar.activation(stq[:, j * CH:(j + 1) * CH], ps[:],
                                         AF.Identity, bias=qkv1b[br])
            dst = cp_qv[br][16 * hi:16 * hi + 16, :].rearrange(
                "h (c w) -> c h w", w=W)
            nc.scalar.dma_start(dst, stq[:].rearrange("c (h w) -> c h w", w=W))

    def qkv2_pass(br):
        sAr = sA[br][:].rearrange("p (h w) -> p w h", w=W)
        for hi in range(8):
            stk = stp.tile([64, 2048], FP16, tag="stk", name=f"stk{br}_{hi}")
            for j in range(4):
                ci = 4 * hi + j
                pst = ps_conv.tile([128, CH], F32, tag="cps", name=f"q2{br}_{hi}_{j}")
                ps = pst[0:64, :]
                nc.tensor.matmul(ps, qkv2w[br],
                                 sAr[:, 4 * ci:4 * ci + 4, :],
                                 start=True, stop=True)
                if hi % 2 == 1:
                    nc.vector.tensor_scalar_add(stk[:, j * CH:(j + 1) * CH],
                                                ps, qkv2b[br])
                else:
                    nc.scalar.activation(stk[:, j * CH:(j + 1) * CH], ps,
                                         AF.Identity, bias=qkv2b[br])
            dst = cp_k[br][16 * hi:16 * hi + 16, :].rearrange(
                "w (c h) -> c w h", h=H)
            nc.scalar.dma_start(dst, stk[:].rearrange("c (w h) -> c w h", h=H))

    g_sb = gsb.tile([128, 16 * 128], FP16, tag="g_sb")
    ps_g = tc.alloc_tile_pool(name="ps_g", bufs=2, space="PSUM")
    ps_conv = tc.alloc_tile_pool(name="ps_conv", bufs=4, space="PSUM")
    ps_bt = tc.alloc_tile_pool(name="ps_bt", bufs=2, space="PSUM")

    _conv_h(tc, nc, wf, pad0, sA[0], ps_conv)
    _conv_w(tc, nc, wf, prr, sA[1], ps_conv)
    qkv1_pass(0)
    qkv2_pass(0)
    qkv1_pass(1)
    pft = {}
    for nh in range(2):
        qa = sprm.tile([128, D * W], FP16, tag="m", name=f"hq{nh}")
        nc.sync.dma_start(qa[:], cp_qv[0][:, nh * D * W:(nh + 1) * D * W])
        va = sprm.tile([128, D * W], FP16, tag="m", name=f"wv{nh}")
        nc.sync.dma_start(
            va[:], cp_qv[1][:, (64 + nh * D) * W:(64 + (nh + 1) * D) * W])
        pft[nh] = (qa, va)
    qkv2_pass(1)

    # ---------------- phase 2: attention ----------------
    def emit_bt(gi):
        # B^T: k loads pipelined one head ahead; gather-stores follow on sync
        ksrc = cp_k[1] if gi == 0 else cp_k[0]
        kts = {}
        def kload(nh):
            ka = sprk.tile([128, D * H], FP16, tag="k", name=f"k{gi}_{nh}")
            nc.sync.dma_start(ka[:], ksrc[:, nh * D * H:(nh + 1) * D * H])
            kts[nh] = ka
        kload(0)
        for nh in range(NH):
            if nh + 1 < NH:
                kload(nh + 1)
            ka = kts.pop(nh)
            gref = g_sb[:, (gi * NH + nh) * 128:(gi * NH + nh + 1) * 128]
            bt = btev.tile([128, D * W], FP16, tag="btv")
            for half in range(2):
                bps = ps_bt.tile([128, CH], F32, tag="bt")
                for j in range(4):
                    d = 4 * half + j
                    nc.tensor.matmul(bps[:, j * 128:(j + 1) * 128],
                                     ka[:, d * H:(d + 1) * H], gref,
                                     start=True, stop=True)
                if half == 0:
                    nc.vector.tensor_copy(bt[:, 0:CH], bps[:])
                else:
                    nc.scalar.activation(bt[:, CH:2 * CH], bps[:], AF.Copy)
            c0 = gi * 64 + nh * D
            nc.sync.dma_start(
                cp_s[c0:c0 + D, :].rearrange("d (h w) -> h d w", w=W),
                bt[:])

    emit_g()
    emit_bt(0)
    emit_bt(1)

    padp.release()
    stp.release()
    scp.release()

    xpf = tc.alloc_tile_pool(name="xpf", bufs=8)
    xfs = []
    ps_bt.release()
    ps_conv.release()

    # ---------------- phase 3: projection + sigmoid + x*sig ----------------
    scp2 = tc.alloc_tile_pool(name="scp2", bufs=1)
    s_cp = scp2.tile([128, HW], FP16, tag="s_cp")
    outp = tc.alloc_tile_pool(name="outp", bufs=4)
    sgp = tc.alloc_tile_pool(name="sgp", bufs=4)
    ps_pj = tc.alloc_tile_pool(name="ps_pj", bufs=4, space="PSUM")

    for hi in range(8):
        nc.sync.dma_start(s_cp[:, hi * 2048:(hi + 1) * 2048],
                          cp_s[:, hi * 2048:(hi + 1) * 2048])
        xc = xpf.tile([64, 2048], F32, tag="xp", name=f"xf{hi}")
        nc.sync.dma_start(xc[:], a["x"][:, hi * 2048:(hi + 1) * 2048])
        xfs.append(xc)
    for hi in range(8):
        xc = xfs[hi]
        yst = outp.tile([64, 2048], F32, tag="yst")
        for j in range(4):
            ci = 4 * hi + j
            pps = ps_pj.tile([64, CH], F32, tag="pj")
            nc.tensor.matmul(pps[:], projw, s_cp[:, ci * CH:(ci + 1) * CH],
                             start=True, stop=True)
            sg = sgp.tile([64, CH], F32, tag="sg")
            nc.scalar.activation(sg[:], pps[:], AF.Sigmoid, bias=projb)
            nc.vector.tensor_mul(yst[:, j * CH:(j + 1) * CH], sg[:],
                                 xc[:, j * CH:(j + 1) * CH])
        nc.scalar.dma_start(a["y"][:, hi * 2048:(hi + 1) * 2048], yst[:])

    for p in (ps_pj, sgp, outp, scp2, xpf, ps_g, btev, gsb, sprk, sprm,
              evp, xcp, wp, dp):
        p.release()


def _prep_weights(inputs):
    """Host-side packing: BN affine, paired conv taps, folded qkv biases."""
    inp = {k: np.asarray(v, dtype=np.float64) for k, v in inputs.items()}
    a1 = inp["bn1_g"] / np.sqrt(inp["bn1_v"] + EPS)
    b1 = inp["bn1_b"] - inp["bn1_m"] * a1
    a2 = inp["bn2_g"] / np.sqrt(inp["bn2_v"] + EPS)
    b2 = inp["bn2_b"] - inp["bn2_m"] * a2

    def conv_pack(ws):
        eff = np.zeros((23, C, C))  # taps 0..20 live; 21,22 stay zero
        for j, k in enumerate(KS):
            off = PAD - k // 2
            for i in range(k):
                eff[off + i] += ws[j][:, :, i]
        pk = np.zeros((128, 704))
        for gi, g in enumerate(L_SET[:-1]):  # 0,1,4,5,8 -> M=128 blocks
            c0 = gi * 128
            pk[0:64, c0:c0 + 64] = eff[2 * g].T
            pk[64:128, c0:c0 + 64] = eff[2 * g + 1].T
            pk[0:64, c0 + 64:c0 + 128] = eff[2 * g + 4].T
            pk[64:128, c0 + 64:c0 + 128] = eff[2 * g + 5].T
        pk[0:64, 640:704] = eff[18].T   # g=9 lower-only block
        pk[64:128, 640:704] = eff[19].T
        return pk

    convh = conv_pack([inp[f"sc1_w{j}"][:, :, :, 0] for j in range(3)])
    convw = conv_pack([inp[f"sc2_w{j}"][:, :, 0, :] for j in range(3)])
    bch = inp["sc1_b0"] + inp["sc1_b1"] + inp["sc1_b2"]
    bcw = inp["sc2_b0"] + inp["sc2_b1"] + inp["sc2_b2"]

    scale = D * H ** (-0.5)
    idx = (np.arange(NH)[:, None] * 24 + np.arange(D)[None, :]).ravel()
    idx_q, idx_k, idx_v = idx, idx + 8, idx + 16

    wf = np.zeros((128, 1856))
    wf[:, 0:704] = convh
    wf[:, 704:1408] = convw
    wa = np.zeros((128, 9))
    wa[0:64, 2] = a1; wa[0:64, 3] = b1
    wa[0:64, 4] = a2; wa[0:64, 5] = b2

    for br, (qw, qb, bc) in enumerate(
            [(inp["hqkv_w"], inp["hqkv_b"], bch),
             (inp["wqkv_w"], inp["wqkv_b"], bcw)]):
        bfold = qb + qw @ bc
        Wq, Wk, Wv = qw[idx_q] * scale, qw[idx_k], qw[idx_v]
        bq, bk, bv = bfold[idx_q] * scale, bfold[idx_k], bfold[idx_v]
        q1 = np.concatenate([Wq.T, Wv.T], axis=1)          # [64, 128]
        wf[0:64, 1408 + 128 * br:1536 + 128 * br] = q1
        wf[64:128, 1408 + 128 * br:1536 + 128 * br] = q1   # dup: sums halves
        wf[0:64, 1664 + 64 * br:1728 + 64 * br] = Wk.T
        wf[64:128, 1664 + 64 * br:1728 + 64 * br] = Wk.T
        wa[:, br] = np.concatenate([bq, bv])
        wa[0:64, 6 + br] = bk

    wf[0:64, 1792:1856] = inp["wout_w"].T
    wf[64:128, 1792:1856] = inp["hout_w"].T
    wa[0:64, 8] = inp["wout_b"] + inp["hout_b"]
    return {"wf": wf.astype(np.float16), "wa": wa.astype(np.float32)}


_NC_CACHE = {}
_RUN_OPTS = {"trace": False}
_LAST_RESULT = {}

_W_SHAPES = {"x": [C, HW], "wf": [128, 1856], "wa": [128, 9]}
_W_DTYPES = {"x": F32, "wf": FP16, "wa": F32}


def _build_nc(reps=1):
    key = f"nc{reps}"
    if key in _NC_CACHE:
        return _NC_CACHE[key]
    nc = bacc.Bacc(trn_type="TRN2", target_bir_lowering=False, debug=False)
    a = {}
    for n, s in _W_SHAPES.items():
        a[n] = nc.dram_tensor(n, s, _W_DTYPES[n], kind="ExternalInput").ap()
    a["y"] = nc.dram_tensor("y", [C, HW], F32, kind="ExternalOutput").ap()
    with tile.TileContext(nc) as tc:
        _kernel_body(tc, a, reps=reps)
    nc.compile()
    _NC_CACHE[key] = nc
    return nc


def _in_maps(inputs):
    w = _prep_weights(inputs)
    x = np.ascontiguousarray(np.asarray(inputs["x"], dtype=np.float32))
    maps = []
    for core in range(N_CORES):
        m = {"x": np.ascontiguousarray(x[core].reshape(C, HW))}
        m.update(w)
        maps.append(m)
    return maps


def kernel(**inputs):
    from concourse.bass_utils import run_bass_kernel_spmd

    nc = _build_nc()
    res = run_bass_kernel_spmd(nc, _in_maps(inputs), core_ids=list(range(N_CORES)),
                               trace=_RUN_OPTS["trace"])
    _LAST_RESULT["res"] = res
    out = np.stack([res.results[i]["y"].reshape(C, H, W) for i in range(N_CORES)])
    return out.astype(np.float32)


if __name__ == "__main__":
    nc = _build_nc()
    print("built ok")
